# revision 1
# baseline (speedup 1.0000x reference)
"""Trainium2 Bass kernel for nn_CrossAttention_16260746183230.

Math (per batch element b; L=2048, E=128):
    w_id, w_q, w_mul = Wc_w[:E], Wc_w[E:2E], Wc_w[2E:]
    S[i,j] = s_id[i] + s_q[j] + sum_e Uid[i,e]*Uq[j,e]*w_mul[e] + Wc_b   (mask == 1)
    P = softmax(S, axis=i)
    A_D2Q = P @ Uq ; A_Q2D = (P @ P^T) @ Uid = P @ (P^T @ Uid)
    Vid = [Uid, A_D2Q, Uid*A_D2Q, Uid*A_Q2D]

Key reductions used by this kernel:
  * softmax over i is invariant to any j-only offset, so the s_q[j] and Wc_b
    terms cancel -- they are never computed.  (mask is all-ones per the input
    spec, so S*mask == S.)
  * s_id[i] folds into the contraction: S'[i,j] = sum_e UidT[e,i]*Yq[e,j]
    with Yq[e,j] = Uq[j,e]*w_mul[e] + w_id[e].
  * P is never normalized explicitly: with E=exp(S'), c[j]=sum_i E[i,j],
        A_D2Q = E @ (Uq / c),   A_Q2D = E @ (T_raw / c^2),
        T_raw = E^T @ Uid
  * A_Q2D via P @ (P^T @ Uid) avoids the [L,L] M matrix (8x fewer FLOPs).

Distribution: pure data-parallel over batch, one batch element per core
(B=8 == n_cores=8), no collectives.  All heavy matmuls run in bf16 with
fp32 PSUM accumulation.
"""

import numpy as np

import concourse.bass as bass
import concourse.tile as tile
from concourse import bacc, mybir
from concourse.bass_utils import run_bass_kernel_spmd
from concourse.masks import make_identity

B, L, E = 8, 2048, 128
NT = L // 128          # 16 tiles of 128 rows
FP = mybir.dt.float32
BF = mybir.dt.bfloat16
Exp = mybir.ActivationFunctionType.Exp
MULT = mybir.AluOpType.mult
ADD = mybir.AluOpType.add


def _emit(tc, nc, uq, uid, wcw, out):
    with (
        tc.tile_pool(name="big", bufs=1) as big,
        tc.tile_pool(name="work", bufs=2) as work,
    ):
        # ---- load inputs -------------------------------------------------
        # [p, t, e] layout: row i = t*128 + p on partition p.
        uq_sb = big.tile([128, NT, E], FP)
        uid_sb = big.tile([128, NT, E], FP)
        nc.sync.dma_start(uq_sb, uq.ap().rearrange("(t p) e -> p t e", p=128))
        nc.sync.dma_start(uid_sb, uid.ap().rearrange("(t p) e -> p t e", p=128))
        w_id = big.tile([128, 1], FP)
        w_mul = big.tile([128, 1], FP)
        nc.sync.dma_start(w_id, wcw.ap()[0:E].rearrange("(p o) -> p o", o=1))
        nc.sync.dma_start(w_mul, wcw.ap()[2 * E:3 * E].rearrange("(p o) -> p o", o=1))

        uq_bf = big.tile([128, NT, E], BF)
        uid_bf = big.tile([128, NT, E], BF)
        nc.vector.tensor_copy(uq_bf, uq_sb)
        nc.vector.tensor_copy(uid_bf, uid_sb)

        ident = big.tile([128, 128], BF)
        make_identity(nc, ident)

        # ---- transposes: uidT[e, i], yq[e, j] = UqT*w_mul + w_id ---------
        uidT = big.tile([128, NT, 128], BF)   # [e, (it, i')]
        yq = big.tile([128, NT, 128], BF)     # [e, (jt, j')]
        with tc.tile_pool(name="ps_tr", bufs=4, space="PSUM") as ps_tr:
            for t in range(NT):
                p1 = ps_tr.tile([128, 128], BF, tag="tr")
                nc.tensor.transpose(p1, uid_bf[:, t, :], ident)
                nc.vector.tensor_copy(uidT[:, t, :], p1)
                p2 = ps_tr.tile([128, 128], BF, tag="tr")
                nc.tensor.transpose(p2, uq_bf[:, t, :], ident)
                nc.vector.tensor_scalar(yq[:, t, :], p2, w_mul, w_id, MULT, ADD)

        # ---- St pass: Et[j, i] = exp(S'[i,j]), c[j] = sum_i -------------
        et = big.tile([128, NT, L], BF)       # 16 x [128 j', 2048 i] = 64KB/part
        chat = big.tile([128, NT], FP)        # c[j] per (j', jt)
        uidT_flat = uidT.rearrange("p t i -> p (t i)")
        yq_flat = yq.rearrange("p t j -> p (t j)")
        with tc.tile_pool(name="ps_st", bufs=2, space="PSUM") as ps_st:
            for jt in range(NT):
                pst = ps_st.tile([128, L], FP, tag="st")
                for c in range(4):
                    nc.tensor.matmul(
                        pst[:, c * 512:(c + 1) * 512],
                        yq[:, jt, :],
                        uidT_flat[:, c * 512:(c + 1) * 512],
                        start=True, stop=True,
                    )
                nc.scalar.activation(
                    et[:, jt, :], pst, Exp, accum_out=chat[:, jt:jt + 1]
                )

        # ---- E pass + T accumulation ------------------------------------
        # E[i, j] recomputed per i-tile (two 1024-halves for PSUM ping-pong);
        # T_raw[j, e] accumulated in PSUM across i-tiles.
        with (
            tc.tile_pool(name="ps_s", bufs=2, space="PSUM") as ps_s,
            tc.tile_pool(name="ps_t", bufs=1, space="PSUM") as ps_t,
        ):
            t_acc = ps_t.tile([128, NT, E], FP)   # [j', (jt, e)] = 4 banks
            for it in range(NT):
                e_half = []
                for h in range(2):
                    psh = ps_s.tile([128, 1024], FP, tag="s")
                    for c in range(2):
                        nc.tensor.matmul(
                            psh[:, c * 512:(c + 1) * 512],
                            uidT[:, it, :],
                            yq_flat[:, h * 1024 + c * 512: h * 1024 + (c + 1) * 512],
                            start=True, stop=True,
                        )
                    eh = work.tile([128, 1024], BF, tag=f"esb{h}")
                    nc.scalar.activation(eh, psh, Exp)
                    e_half.append(eh)
                for jt in range(NT):
                    nc.tensor.matmul(
                        t_acc[:, jt, :],
                        e_half[jt // 8][:, (jt % 8) * 128:(jt % 8 + 1) * 128],
                        uid_bf[:, it, :],
                        start=(it == 0), stop=(it == NT - 1),
                    )

            # ---- normalization factors ----------------------------------
            rcol = big.tile([128, NT], FP)
            r2col = big.tile([128, NT], FP)
            nc.vector.reciprocal(rcol, chat)
            nc.vector.tensor_mul(r2col, rcol, rcol)
            # uqtp[:, jt, 0:E] = Uq'[j,e] = Uq/c ; [:, jt, E:2E] = T' = T_raw/c^2
            uqtp = big.tile([128, NT, 2 * E], BF)
            for jt in range(NT):
                nc.vector.tensor_scalar_mul(
                    uqtp[:, jt, 0:E], uq_sb[:, jt, :], rcol[:, jt:jt + 1]
                )
                nc.vector.tensor_scalar_mul(
                    uqtp[:, jt, E:2 * E], t_acc[:, jt, :], r2col[:, jt:jt + 1]
                )

        # ---- A pass: [A_D2Q | A_Q2D] and output assembly ----------------
        out_t = out.ap().rearrange("(t p) c -> t p c", p=128)
        with tc.tile_pool(name="ps_a", bufs=4, space="PSUM") as ps_a:
            for it in range(NT):
                a12 = ps_a.tile([128, 2 * E], FP, tag="a")
                for jt in range(NT):
                    nc.tensor.matmul(
                        a12,
                        et[:, jt, it * 128:(it + 1) * 128],
                        uqtp[:, jt, :],
                        start=(jt == 0), stop=(jt == NT - 1),
                    )
                v = work.tile([128, 4 * E], FP, tag="v")
                nc.vector.tensor_copy(v[:, 0:E], uid_sb[:, it, :])
                nc.scalar.copy(v[:, E:2 * E], a12[:, 0:E])
                nc.vector.tensor_mul(v[:, 2 * E:3 * E], uid_sb[:, it, :], a12[:, 0:E])
                nc.vector.tensor_mul(v[:, 3 * E:4 * E], uid_sb[:, it, :], a12[:, E:2 * E])
                nc.sync.dma_start(out_t[it], v)


def build():
    nc = bacc.Bacc("TRN2", target_bir_lowering=False, debug=False)
    uq = nc.dram_tensor("uq", [L, E], FP, kind="ExternalInput")
    uid = nc.dram_tensor("uid", [L, E], FP, kind="ExternalInput")
    wcw = nc.dram_tensor("wcw", [3 * E], FP, kind="ExternalInput")
    out = nc.dram_tensor("out", [L, 4 * E], FP, kind="ExternalOutput")
    with tile.TileContext(nc) as tc:
        _emit(tc, nc, uq, uid, wcw, out)
    nc.compile()
    return nc


_nc_cache = None


def _get_nc():
    global _nc_cache
    if _nc_cache is None:
        _nc_cache = build()
    return _nc_cache


def kernel(Uq, Uid, mask, Wc_w, Wc_b, **_unused):
    """Full inputs in, full output out.  Shards batch across 8 NeuronCores."""
    Uq = np.ascontiguousarray(np.asarray(Uq, dtype=np.float32))
    Uid = np.ascontiguousarray(np.asarray(Uid, dtype=np.float32))
    Wc_w = np.ascontiguousarray(np.asarray(Wc_w, dtype=np.float32))
    nc = _get_nc()
    in_maps = [
        {"uq": Uq[b], "uid": Uid[b], "wcw": Wc_w}
        for b in range(B)
    ]
    res = run_bass_kernel_spmd(nc, in_maps, core_ids=list(range(B)))
    return np.stack([res.results[b]["out"] for b in range(B)], axis=0)


# revision 4
# speedup vs baseline: 27.6664x; 27.6664x over previous
"""Trainium2 Bass kernel for nn_CrossAttention_16260746183230.

Math (per batch element b; L=2048, E=128):
    w_id, w_q, w_mul = Wc_w[:E], Wc_w[E:2E], Wc_w[2E:]
    S[i,j] = s_id[i] + s_q[j] + sum_e Uid[i,e]*Uq[j,e]*w_mul[e] + Wc_b   (mask == 1)
    P = softmax(S, axis=i)
    A_D2Q = P @ Uq ; A_Q2D = (P @ P^T) @ Uid = P @ (P^T @ Uid)
    Vid = [Uid, A_D2Q, Uid*A_D2Q, Uid*A_Q2D]

Key reductions used by this kernel:
  * softmax over i is invariant to any j-only offset, so the s_q[j] and Wc_b
    terms cancel -- they are never computed.  (mask is all-ones per the input
    spec, so S*mask == S.)
  * s_id[i] folds into the contraction: S'[i,j] = sum_e UidT[e,i]*Yq[e,j]
    with Yq[e,j] = Uq[j,e]*w_mul[e] + w_id[e].
  * P is never normalized explicitly: with E=exp(S'), c[j]=sum_i E[i,j],
        A_D2Q = E @ (Uq / c),   A_Q2D = E @ (T_raw / c^2),
        T_raw = E^T @ Uid
  * A_Q2D via P @ (P^T @ Uid) avoids the [L,L] M matrix (8x fewer FLOPs).

Distribution: pure data-parallel over batch, one batch element per core
(B=8 == n_cores=8), no collectives.  All heavy matmuls run in bf16 with
fp32 PSUM accumulation.
"""

import numpy as np

import concourse.bass as bass
import concourse.tile as tile
from concourse import bacc, mybir
from concourse.bass_utils import run_bass_kernel_spmd
from concourse.masks import make_identity

B, L, E = 8, 2048, 128
NT = L // 128          # 16 tiles of 128 rows
FP = mybir.dt.float32
BF = mybir.dt.bfloat16
Exp = mybir.ActivationFunctionType.Exp
MULT = mybir.AluOpType.mult
ADD = mybir.AluOpType.add


def _emit(tc, nc, uq, uid, wcw, out):
    with (
        tc.tile_pool(name="big", bufs=1) as big,
        tc.tile_pool(name="work", bufs=2) as work,
    ):
        # ---- load inputs -------------------------------------------------
        # [p, t, e] layout: row i = t*128 + p on partition p.
        uq_sb = big.tile([128, NT, E], FP)
        uid_sb = big.tile([128, NT, E], FP)
        nc.sync.dma_start(uq_sb, uq.ap().rearrange("(t p) e -> p t e", p=128))
        nc.sync.dma_start(uid_sb, uid.ap().rearrange("(t p) e -> p t e", p=128))
        w_id = big.tile([128, 1], FP)
        w_mul = big.tile([128, 1], FP)
        nc.sync.dma_start(w_id, wcw.ap()[0:E].rearrange("(p o) -> p o", o=1))
        nc.sync.dma_start(w_mul, wcw.ap()[2 * E:3 * E].rearrange("(p o) -> p o", o=1))

        uq_bf = big.tile([128, NT, E], BF)
        uid_bf = big.tile([128, NT, E], BF)
        nc.vector.tensor_copy(uq_bf, uq_sb)
        nc.vector.tensor_copy(uid_bf, uid_sb)

        ident = big.tile([128, 128], BF)
        make_identity(nc, ident)

        # ---- transposes: uidT[e, i], yq[e, j] = UqT*w_mul + w_id ---------
        uidT = big.tile([128, NT, 128], BF)   # [e, (it, i')]
        yq = big.tile([128, NT, 128], BF)     # [e, (jt, j')]
        with tc.tile_pool(name="ps_tr", bufs=4, space="PSUM") as ps_tr:
            for t in range(NT):
                p1 = ps_tr.tile([128, 128], BF, tag="tr")
                nc.tensor.transpose(p1, uid_bf[:, t, :], ident)
                nc.vector.tensor_copy(uidT[:, t, :], p1)
                p2 = ps_tr.tile([128, 128], BF, tag="tr")
                nc.tensor.transpose(p2, uq_bf[:, t, :], ident)
                nc.vector.tensor_scalar(yq[:, t, :], p2, w_mul, w_id, MULT, ADD)

        # ---- St pass: Et[j, i] = exp(S'[i,j]), c[j] = sum_i -------------
        et = big.tile([128, NT, L], BF)       # 16 x [128 j', 2048 i] = 64KB/part
        chat = big.tile([128, NT], FP)        # c[j] per (j', jt)
        uidT_flat = uidT.rearrange("p t i -> p (t i)")
        yq_flat = yq.rearrange("p t j -> p (t j)")
        with tc.tile_pool(name="ps_st", bufs=2, space="PSUM") as ps_st:
            for jt in range(NT):
                pst = ps_st.tile([128, L], FP, tag="st")
                for c in range(4):
                    nc.tensor.matmul(
                        pst[:, c * 512:(c + 1) * 512],
                        yq[:, jt, :],
                        uidT_flat[:, c * 512:(c + 1) * 512],
                        start=True, stop=True,
                    )
                nc.scalar.activation(
                    et[:, jt, :], pst, Exp, accum_out=chat[:, jt:jt + 1]
                )

        # ---- E pass + T accumulation ------------------------------------
        # E[i, j] recomputed per i-tile (two 1024-halves for PSUM ping-pong);
        # T_raw[j, e] accumulated in PSUM across i-tiles.
        with (
            tc.tile_pool(name="ps_s", bufs=2, space="PSUM") as ps_s,
            tc.tile_pool(name="ps_t", bufs=1, space="PSUM") as ps_t,
        ):
            t_acc = ps_t.tile([128, NT, E], FP)   # [j', (jt, e)] = 4 banks
            for it in range(NT):
                e_half = []
                for h in range(2):
                    psh = ps_s.tile([128, 1024], FP, tag="s")
                    for c in range(2):
                        nc.tensor.matmul(
                            psh[:, c * 512:(c + 1) * 512],
                            uidT[:, it, :],
                            yq_flat[:, h * 1024 + c * 512: h * 1024 + (c + 1) * 512],
                            start=True, stop=True,
                        )
                    eh = work.tile([128, 1024], BF, tag=f"esb{h}")
                    nc.scalar.activation(eh, psh, Exp)
                    e_half.append(eh)
                for jt in range(NT):
                    nc.tensor.matmul(
                        t_acc[:, jt, :],
                        e_half[jt // 8][:, (jt % 8) * 128:(jt % 8 + 1) * 128],
                        uid_bf[:, it, :],
                        start=(it == 0), stop=(it == NT - 1),
                    )

            # ---- normalization factors ----------------------------------
            rcol = big.tile([128, NT], FP)
            r2col = big.tile([128, NT], FP)
            nc.vector.reciprocal(rcol, chat)
            nc.vector.tensor_mul(r2col, rcol, rcol)
            # uqtp[:, jt, 0:E] = Uq'[j,e] = Uq/c ; [:, jt, E:2E] = T' = T_raw/c^2
            uqtp = big.tile([128, NT, 2 * E], BF)
            for jt in range(NT):
                nc.vector.tensor_scalar_mul(
                    uqtp[:, jt, 0:E], uq_sb[:, jt, :], rcol[:, jt:jt + 1]
                )
                nc.vector.tensor_scalar_mul(
                    uqtp[:, jt, E:2 * E], t_acc[:, jt, :], r2col[:, jt:jt + 1]
                )

        # ---- A pass: [A_D2Q | A_Q2D] and output assembly ----------------
        out_t = out.ap().rearrange("(t p) c -> t p c", p=128)
        with tc.tile_pool(name="ps_a", bufs=4, space="PSUM") as ps_a:
            for it in range(NT):
                a12 = ps_a.tile([128, 2 * E], FP, tag="a")
                for jt in range(NT):
                    nc.tensor.matmul(
                        a12,
                        et[:, jt, it * 128:(it + 1) * 128],
                        uqtp[:, jt, :],
                        start=(jt == 0), stop=(jt == NT - 1),
                    )
                v = work.tile([128, 4 * E], FP, tag="v")
                nc.vector.tensor_copy(v[:, 0:E], uid_sb[:, it, :])
                nc.scalar.copy(v[:, E:2 * E], a12[:, 0:E])
                nc.vector.tensor_mul(v[:, 2 * E:3 * E], uid_sb[:, it, :], a12[:, 0:E])
                nc.vector.tensor_mul(v[:, 3 * E:4 * E], uid_sb[:, it, :], a12[:, E:2 * E])
                nc.sync.dma_start(out_t[it], v)


def build(reps=1):
    nc = bacc.Bacc("TRN2", target_bir_lowering=False, debug=False)
    uq = nc.dram_tensor("uq", [L, E], FP, kind="ExternalInput")
    uid = nc.dram_tensor("uid", [L, E], FP, kind="ExternalInput")
    wcw = nc.dram_tensor("wcw", [3 * E], FP, kind="ExternalInput")
    out = nc.dram_tensor("out", [L, 4 * E], FP, kind="ExternalOutput")
    with tile.TileContext(nc) as tc:
        for _ in range(reps):
            _emit(tc, nc, uq, uid, wcw, out)
    nc.compile()
    return nc


_nc_cache = None


def _get_nc():
    global _nc_cache
    if _nc_cache is None:
        _nc_cache = build()
    return _nc_cache


def kernel(Uq, Uid, mask, Wc_w, Wc_b, **_unused):
    """Full inputs in, full output out.  Shards batch across 8 NeuronCores."""
    Uq = np.ascontiguousarray(np.asarray(Uq, dtype=np.float32))
    Uid = np.ascontiguousarray(np.asarray(Uid, dtype=np.float32))
    Wc_w = np.ascontiguousarray(np.asarray(Wc_w, dtype=np.float32))
    nc = _get_nc()
    in_maps = [
        {"uq": Uq[b], "uid": Uid[b], "wcw": Wc_w}
        for b in range(B)
    ]
    res = run_bass_kernel_spmd(nc, in_maps, core_ids=list(range(B)))
    return np.stack([res.results[b]["out"] for b in range(B)], axis=0)
